# revision 1
# baseline (speedup 1.0000x reference)
"""Trainium2 Bass kernel for nn_JointLoss (recon MSE + SimCLR-style contrastive + group distance loss).

Strategy (data-parallel over 8 NeuronCores):
  - Each core owns a 1024-row block of the 8192x8192 similarity matrix.
  - Each core receives a row-ROTATED copy of projections (np.roll by -c*1024) so
    its own rows sit at local indices 0..1023 -> positive-block offsets are
    core-independent and the NEFF is pure SPMD.
  - On device: PE transposes P (fp32, via identity matmul) into a bf16 P^T
    [128 x 8192]; 128 bf16 matmuls (N=512) stream sim chunks into a single
    8-bank PSUM tensor; ScalarE does exp(10*x) IN-PLACE on PSUM in 2048-wide
    chunks with accum_out row-sums; VectorE computes masked group sums
    (positives), recon-MSE partials and distance-loss partials.
  - Device outputs per core are tiny: rowsum[128,8], possum[128,8], partials[1,4].
  - Host finishes in float64: closs = mean(log(rowsum)-log(possum)), etc.
"""

import sys

if "/opt/trn_rl_repo" not in sys.path:
    sys.path.insert(0, "/opt/trn_rl_repo")

from contextlib import ExitStack

import numpy as np

import concourse.bacc as bacc
import concourse.bass_isa as bass_isa
import concourse.tile as tile
from concourse import mybir
from concourse.bass_utils import run_bass_kernel_spmd

N = 8192
D = 128
F = 784
NCORES = 8
RPC = N // NCORES  # 1024 rows per core
RT = RPC // 128    # 8 row-tiles per core
NT = N // 128      # 64 transpose tiles
NQ = 4             # column quarters (2048 cols each)
TAU = 0.1

f32 = mybir.dt.float32
bf16 = mybir.dt.bfloat16


import os

_STAGE = int(os.environ.get("KERNEL_STAGE", "99"))  # debug bisect knob


def _kernel_body(tc, proj, xr, rl, ident, mask, rowsum_o, possum_o, partials_o):
    nc = tc.nc
    AX = mybir.AxisListType
    ALU = mybir.AluOpType
    with ExitStack() as ctx:
        consts = ctx.enter_context(tc.tile_pool(name="consts", bufs=1))
        big = ctx.enter_context(tc.tile_pool(name="big", bufs=1))
        ptin = ctx.enter_context(tc.tile_pool(name="ptin", bufs=4))
        qbp = ctx.enter_context(tc.tile_pool(name="qbp", bufs=2))
        dpool = ctx.enter_context(tc.tile_pool(name="dpool", bufs=3))
        stats = ctx.enter_context(tc.tile_pool(name="stats", bufs=1))
        psum = ctx.enter_context(tc.tile_pool(name="psum", bufs=1, space="PSUM"))

        ident_sb = consts.tile([128, 128], f32)
        nc.scalar.dma_start(ident_sb, ident)
        mask_sb = consts.tile([128, 128], f32)
        nc.scalar.dma_start(mask_sb, mask)
        identb = consts.tile([128, 128], bf16)
        nc.vector.tensor_copy(identb, ident_sb)

        pt_bf = big.tile([128, N], bf16)     # full P^T in bf16
        # proj quarters first on the sync ring (critical path), then xr/rl
        # behind them on the same FIFO so they can't steal DMA bandwidth
        pt_ins = []
        for q in range(NQ):
            t = ptin.tile([128, NT // NQ, 128], f32, tag="ptiles")
            nc.sync.dma_start(t, proj.rearrange("(q t p) d -> q p t d", q=NQ, p=128)[q])
            pt_ins.append(t)
        xr_sb = big.tile([128, RT, F], f32)
        nc.sync.dma_start(xr_sb, xr.rearrange("(t p) j -> p t j", p=128))
        rl_sb = big.tile([128, RT, F], f32)
        nc.sync.dma_start(rl_sb, rl.rearrange("(t p) j -> p t j", p=128))

        rowsum_parts = stats.tile([128, RT, NQ], f32)
        rowsum_sb = stats.tile([128, RT], f32)
        possum_sb = stats.tile([128, RT], f32)
        recon_parts = stats.tile([128, RT], f32)
        s_groups = stats.tile([128, RPC // 4], f32)
        junk1024 = stats.tile([128, RPC], f32)
        stats4 = stats.tile([128, 4], f32)
        partials_sb = stats.tile([1, 4], f32)

        if _STAGE < 99:
            nc.vector.memset(rowsum_parts, 1.0)
            nc.vector.memset(possum_sb, 1.0)
        if _STAGE < 1:
            nc.vector.memset(pt_own, 0.0)
            nc.vector.memset(pt_bf, 0.0)

        pacc = psum.tile([128, 4096], f32)  # all 8 PSUM banks

        proj_q = proj.rearrange("(q t p) d -> q p t d", q=NQ, p=128)

        half = 0
        for q in range(NQ):
            pt_in = pt_ins[q]
            qb = qbp.tile([128, NT // NQ, 128], bf16, tag="qb")
            nc.vector.tensor_copy(qb, pt_in)
            # bf16 transposes for this quarter's 16 column tiles (1 cyc/col)
            for tl in range(NT // NQ):
                t = q * (NT // NQ) + tl
                slot = t % 8
                pslice = pacc[:, slot * 512 : slot * 512 + 64].bitcast(bf16)
                if _STAGE < 1:
                    continue
                nc.tensor.transpose(pslice, qb[:, tl, :], identb)
                nc.vector.tensor_copy(pt_bf[:, t * 128 : (t + 1) * 128], pslice)
            if _STAGE < 1:
                continue
            # matmuls + exp for this quarter
            for rt in range(RT):
                w = pt_bf[:, rt * 128 : (rt + 1) * 128]
                base = half * 2048
                if _STAGE < 2:
                    continue
                for j in range(4):
                    nc.tensor.matmul(
                        pacc[:, base + j * 512 : base + (j + 1) * 512],
                        w,
                        pt_bf[:, q * 2048 + j * 512 : q * 2048 + (j + 1) * 512],
                        start=True,
                        stop=True,
                    )
                if _STAGE < 3:
                    continue
                if _STAGE >= 4:
                    nc.scalar.activation(
                        pacc[:, base : base + 2048],
                        pacc[:, base : base + 2048],
                        mybir.ActivationFunctionType.Exp,
                        scale=1.0 / TAU,
                        accum_out=rowsum_parts[:, rt, q : q + 1],
                    )
                if q == 0 and _STAGE >= 4:
                    # possum from the exp'd diagonal block still in PSUM
                    pj = dpool.tile([128, 128], f32, tag="pjunk")
                    nc.vector.tensor_mul(
                        pj, pacc[:, base + rt * 128 : base + rt * 128 + 128], mask_sb
                    )
                    nc.vector.reduce_sum(
                        possum_sb[:, rt : rt + 1], pj, axis=AX.X
                    )
                half ^= 1
            # interleave MSE / dist-loss DVE work into quarter slack so it
            # doesn't extend the tail after the last exp chunk
            if q == 1:
                for t in range(4):
                    dtile = dpool.tile([128, F], f32, tag="d")
                    nc.vector.tensor_sub(dtile, xr_sb[:, t, :], rl_sb[:, t, :])
                    nc.vector.tensor_mul(dtile, dtile, dtile)
                    nc.vector.reduce_sum(recon_parts[:, t : t + 1], dtile, axis=AX.X)
            if q == 2:
                for t in range(4, RT):
                    dtile = dpool.tile([128, F], f32, tag="d")
                    nc.vector.tensor_sub(dtile, xr_sb[:, t, :], rl_sb[:, t, :])
                    nc.vector.tensor_mul(dtile, dtile, dtile)
                    nc.vector.reduce_sum(recon_parts[:, t : t + 1], dtile, axis=AX.X)
                nc.vector.reduce_sum(stats4[:, 0:1], recon_parts, axis=AX.X)
                pb_own = pt_bf[:, 0:RPC]
                nc.vector.reduce_sum(
                    s_groups, pb_own.rearrange("p (g s) -> p g s", s=4), axis=AX.X
                )
                nc.vector.tensor_mul(junk1024, pb_own, pb_own)
                nc.vector.reduce_sum(stats4[:, 1:2], junk1024, axis=AX.X)
                nc.vector.tensor_mul(junk1024[:, : RPC // 4], s_groups, s_groups)
                nc.vector.reduce_sum(stats4[:, 2:3], junk1024[:, : RPC // 4], axis=AX.X)
                nc.vector.memset(stats4[:, 3:4], 0.0)

        # rowsum over quarters
        nc.vector.reduce_sum(rowsum_sb, rowsum_parts, axis=AX.X)

        nc.sync.dma_start(partials_o, stats4)
        nc.sync.dma_start(rowsum_o, rowsum_sb)
        nc.sync.dma_start(possum_o, possum_sb)


def _build():
    nc = bacc.Bacc("TRN2", target_bir_lowering=False, debug=False, num_devices=NCORES)
    proj = nc.dram_tensor("proj", [N, D], f32, kind="ExternalInput").ap()
    xr = nc.dram_tensor("xr", [RPC, F], f32, kind="ExternalInput").ap()
    rl = nc.dram_tensor("rl", [RPC, F], f32, kind="ExternalInput").ap()
    ident = nc.dram_tensor("ident", [128, 128], f32, kind="ExternalInput").ap()
    mask = nc.dram_tensor("mask", [128, 128], f32, kind="ExternalInput").ap()
    rowsum_o = nc.dram_tensor("rowsum_o", [128, RT], f32, kind="ExternalOutput").ap()
    possum_o = nc.dram_tensor("possum_o", [128, RT], f32, kind="ExternalOutput").ap()
    partials_o = nc.dram_tensor("partials_o", [128, 4], f32, kind="ExternalOutput").ap()

    with tile.TileContext(nc) as tc:
        _kernel_body(tc, proj, xr, rl, ident, mask, rowsum_o, possum_o, partials_o)
    nc.compile()
    return nc


_NC_CACHE = None


def _get_nc():
    global _NC_CACHE
    if _NC_CACHE is None:
        _NC_CACHE = _build()
    return _NC_CACHE


def _run(projections, xrecon, recon_label, trace=False, **spmd_kwargs):
    nc = _get_nc()
    P = np.ascontiguousarray(np.asarray(projections, dtype=np.float32))
    XR = np.ascontiguousarray(np.asarray(xrecon, dtype=np.float32))
    RL = np.ascontiguousarray(np.asarray(recon_label, dtype=np.float32))
    ident = np.eye(128, dtype=np.float32)
    mask = np.kron(np.eye(32, dtype=np.float32), np.ones((4, 4), dtype=np.float32))
    in_maps = []
    for c in range(NCORES):
        in_maps.append(
            {
                "proj": np.ascontiguousarray(np.roll(P, -c * RPC, axis=0)),
                "xr": np.ascontiguousarray(XR[c * RPC : (c + 1) * RPC]),
                "rl": np.ascontiguousarray(RL[c * RPC : (c + 1) * RPC]),
                "ident": ident,
                "mask": mask,
            }
        )
    return run_bass_kernel_spmd(
        nc, in_maps, core_ids=list(range(NCORES)), trace=trace, **spmd_kwargs
    )


def _combine(results):
    rowsum = np.concatenate(
        [results[c]["rowsum_o"].T.reshape(-1) for c in range(NCORES)]
    ).astype(np.float64)
    possum = np.concatenate(
        [results[c]["possum_o"].T.reshape(-1) for c in range(NCORES)]
    ).astype(np.float64)
    recon_ss = sum(float(results[c]["partials_o"][:, 0].astype(np.float64).sum()) for c in range(NCORES))
    A = sum(float(results[c]["partials_o"][:, 1].astype(np.float64).sum()) for c in range(NCORES))
    B = sum(float(results[c]["partials_o"][:, 2].astype(np.float64).sum()) for c in range(NCORES))
    closs = float(np.mean(np.log(rowsum) - np.log(possum)))
    recon_loss = recon_ss / (N * F)
    dist_loss = (4.0 * A - B) / ((N // 4) * 6 * D)
    loss = closs + recon_loss + dist_loss
    return (
        np.float32(loss),
        np.float32(closs),
        np.float32(recon_loss),
        np.float32(dist_loss),
    )


def kernel(projections, xrecon, recon_label):
    br = _run(projections, xrecon, recon_label)
    return _combine(br.results)



# revision 31
# speedup vs baseline: 1.1945x; 1.1945x over previous
"""Trainium2 Bass kernel for nn_JointLoss (recon MSE + SimCLR contrastive + group distance loss).

Strategy (symmetric data-parallel over 8 NeuronCores):
  - exp(sim) dominates: the ACT engine is the only engine with exp and runs
    1 col/cycle @1.2GHz.  The sim matrix is symmetric, so each core only
    evaluates its rows vs column groups c..c+3 (full) and c+4 (3 quarters:
    (0,0),(0,1),(1,1) of the 1024x1024 pair block) -- 38,912 cols per core
    instead of 65,536.  Transposed entries are recovered as COLUMN sums of
    evaluated blocks (ones-vector matmuls on the PE accumulated in PSUM
    bank 7), combined on the host.
  - All inputs are pre-cast to bf16 on the host (halves DMA traffic).
  - Per core: P^T built via PE transposes (bank 7, before colsums need it);
    sim = bf16 matmuls into PSUM regions A/B ([0:1792]/[1792:3584],
    ping-pong); ACT does exp(10x) PSUM->SBUF bf16 in 24 activations with
    accum_out giving row-sums (DVE reduce-accum ops crash on this HW with
    bf16; plain DVE reduces run ~1.5ns/col - both too slow/broken, so ACT
    accumulates).  GpSimd does the recon-MSE subtract; DVE does possum,
    affine_mul_reduce for MSE/dist partials, pt copies and colsum drains.
  - Host finishes in float64 from tiny outputs.
"""

import os
import sys

if "/opt/trn_rl_repo" not in sys.path:
    sys.path.insert(0, "/opt/trn_rl_repo")

from contextlib import ExitStack

import numpy as np
import ml_dtypes

import concourse.bacc as bacc
import concourse.tile as tile
from concourse import mybir
from concourse.bass_utils import run_bass_kernel_spmd

N = 8192
D = 128
F = 784
NCORES = 8
RPC = N // NCORES   # 1024 rows per core
RT = RPC // 128     # 8 row tiles
PROJ_ROWS = 5 * RPC  # 5120: groups c..c+4
TAU = 0.1

f32 = mybir.dt.float32
bf16 = mybir.dt.bfloat16
BF = ml_dtypes.bfloat16

# colsum chunks: (local col start, n tiles summed, bank-7 partition, schedule)
# k=0..5 cover cols [1024:4096] for all 8 tiles; k=6 covers [4608:5120] for
# tiles 0-3 (the (0,1) quarter of the distance-4 pair block).
COL_CHUNKS = [
    (1024, 8, 0),   # burst during G1 pass steps 0-1
    (1536, 8, 0),   # G2 pass t0-1
    (2048, 8, 32),  # G2 pass t2-3
    (2560, 8, 0),   # G2 pass t4-5
    (3072, 8, 32),  # G2 pass t6-7
    (3584, 8, 64),  # chase G2 acts (lag 1)
    (4608, 4, 32),  # burst at G2 pass t4 (tiles 0-3 acts done)
]

# act-group grid: G0 [0:1792], G1 [1792:3584], G2 [3584:5120] (t<4) or
# [3584:4096]+[4608:5120] packed at [3584:4608] (t>=4)
GB0, GB1, GB2 = 1792, 3584, 5120


def _bank_splits(a, b):
    """split psum range [a,b) at 512 boundaries"""
    cuts = [a]
    nxt = (a // 512 + 1) * 512
    while nxt < b:
        cuts.append(nxt)
        nxt += 512
    cuts.append(b)
    return list(zip(cuts[:-1], cuts[1:]))


def _kernel_body(tc, proj, xr, rl, ident, mask, rowsum_o, possum_o, colsum_o, partials_o):
    nc = tc.nc
    AX = mybir.AxisListType
    ALU = mybir.AluOpType
    EXP = mybir.ActivationFunctionType.Exp
    with ExitStack() as ctx:
        consts = ctx.enter_context(tc.tile_pool(name="consts", bufs=1))
        big = ctx.enter_context(tc.tile_pool(name="big", bufs=1))
        stage = ctx.enter_context(tc.tile_pool(name="stage", bufs=5))
        stats = ctx.enter_context(tc.tile_pool(name="stats", bufs=1))
        psum = ctx.enter_context(tc.tile_pool(name="psum", bufs=1, space="PSUM"))

        identb = consts.tile([128, 128], bf16)
        nc.sync.dma_start(identb, ident)
        maskb = consts.tile([128, 128], bf16)
        nc.sync.dma_start(maskb, mask)
        ones_bf = consts.tile([128, 1], bf16)
        nc.vector.memset(ones_bf, 1.0)

        pt_bf = big.tile([128, PROJ_ROWS], bf16)          # P^T  (d-major)
        exp_sbuf = big.tile([128, RT, PROJ_ROWS], bf16)   # exp(sim) rows x cols
        xr_sb = big.tile([128, RT, F], bf16)
        rl_sb = big.tile([128, RT, F], bf16)

        # proj DMA in 5 rounds of 1024 rows (8 x 128-row transpose tiles each)
        stg = []
        for r in range(5):
            t = stage.tile([128, 8, 128], bf16, tag="pstage")
            nc.sync.dma_start(
                t, proj[r * 1024 : (r + 1) * 1024].rearrange("(t p) d -> p t d", p=128)
            )
            stg.append(t)
        nc.sync.dma_start(xr_sb, xr.rearrange("(t p) j -> p t j", p=128))
        nc.sync.dma_start(rl_sb, rl.rearrange("(t p) j -> p t j", p=128))

        rparts = stats.tile([128, RT, 3], f32)       # per-(tile, act) rowsum partials
        rowsum_sb = stats.tile([128, RT], f32)
        possum_sb = stats.tile([128, RT], f32)
        recon_parts = stats.tile([128, RT], f32)
        stats4 = stats.tile([128, 4], f32)
        s_groups = stats.tile([128, RPC // 4], f32)
        junk_bf = stats.tile([128, RPC], bf16)
        junkf = stats.tile([128, RPC // 4], f32)
        d_bf0 = stats.tile([128, F], bf16)
        d_bf1 = stats.tile([128, F], bf16)
        d_bf = [d_bf0, d_bf1]
        pj = stats.tile([128, 128], bf16)
        colstage = stats.tile([128, 7, 512], f32)

        pacc = psum.tile([128, 4096], f32)  # 8 banks
        # regions A [0:1792] / B [1792:3584]; bank 7 [3584:4096]:
        # transpose slots (early) then colsum chains (from G1 pass on)
        REG = [0, 1792]

        def transpose_round(r):
            # 8 transposes of stage round r -> bank 7 -> copy to pt cols [1024r:1024r+1024]
            for j in range(8):
                slot = pacc[:, 3584 + j * 64 : 3584 + (j + 1) * 64].bitcast(bf16)
                nc.tensor.transpose(slot, stg[r][:, j, :], identb)
            nc.vector.tensor_copy(
                pt_bf[:, r * 1024 : (r + 1) * 1024],
                pacc[:, 3584 : 3584 + 512].bitcast(bf16),
            )

        def colsum_mm(k, t):
            ck, ntiles, part = COL_CHUNKS[k]
            nc.tensor.matmul(
                pacc[part : part + 1, 3584:4096],
                ones_bf[:, 0:1],
                exp_sbuf[:, t, ck : ck + 512],
                start=(t == 0),
                stop=(t == ntiles - 1),
            )

        def colsum_drain(k):
            part = COL_CHUNKS[k][2]
            nc.vector.tensor_copy(
                colstage[part : part + 1, k, :], pacc[part : part + 1, 3584:4096]
            )

        def sim_act(t, au, g):
            # matmuls + one exp act for act-group g of tile t into region au%2.
            # returns nothing; accum_out -> rparts[:, t, g]
            reg = REG[au % 2]
            w = pt_bf[:, t * 128 : (t + 1) * 128]
            if g == 0:
                cols = [(0, GB0)]
            elif g == 1:
                cols = [(GB0, GB1)]
            elif t < 4:
                cols = [(GB1, GB2)]
            else:
                cols = [(GB1, 4096), (4608, 5120)]
            off = 0
            for c0, c1 in cols:
                for p0, p1 in _bank_splits(reg + off, reg + off + (c1 - c0)):
                    s0 = c0 + (p0 - reg - off)
                    nc.tensor.matmul(
                        pacc[:, p0:p1], w, pt_bf[:, s0 : s0 + (p1 - p0)],
                        start=True, stop=True,
                    )
                off += c1 - c0
            # output column range (packed for t>=4 g2)
            if g == 0:
                o0, o1 = 0, GB0
            elif g == 1:
                o0, o1 = GB0, GB1
            elif t < 4:
                o0, o1 = GB1, GB2
            else:
                o0, o1 = GB1, 4608
            nc.scalar.activation(
                exp_sbuf[:, t, o0:o1],
                pacc[:, reg : reg + (o1 - o0)],
                EXP,
                scale=1.0 / TAU,
                accum_out=rparts[:, t, g : g + 1],
            )

        # prologue: transpose rounds 0-1 (pt cols 0:2048)
        transpose_round(0)
        transpose_round(1)

        # distance-loss partials (DVE) from pt cols [0:1024] (own rows)
        pb = pt_bf[:, 0:RPC]
        nc.vector.reduce_sum(s_groups, pb.rearrange("p (g s) -> p g s", s=4), axis=AX.X)
        nc.vector.affine_mul_reduce(
            out=junk_bf, accum_out=stats4[:, 1:2], in0=pb, in1=pb, scale=1.0, bias=0.0
        )
        nc.vector.tensor_tensor(junkf, s_groups, s_groups, ALU.mult)
        nc.vector.reduce_sum(stats4[:, 2:3], junkf, axis=AX.X)
        nc.vector.memset(stats4[:, 3:4], 0.0)

        au = 0

        # ---- G0 pass: cols [0:1792] ----
        for t in range(RT):
            sim_act(t, au, 0)
            au += 1
            # possum from the diagonal 128-block
            nc.vector.tensor_tensor(
                pj, exp_sbuf[:, t, t * 128 : (t + 1) * 128], maskb, ALU.mult
            )
            nc.vector.reduce_sum(possum_sb[:, t : t + 1], pj, axis=AX.X)
            if t == 1:
                transpose_round(2)
            if t == 3:
                transpose_round(3)
            if t == 5:
                transpose_round(4)

        # ---- G1 pass: cols [1792:3584]; colsum chunk 0; MSE ----
        for t in range(RT):
            sim_act(t, au, 1)
            au += 1
            if t < 2:
                for tt in range(4 * t, 4 * t + 4):
                    colsum_mm(0, tt)
                if t == 1:
                    colsum_drain(0)
            # MSE tile t: Pool subtract, DVE fused square+reduce
            nc.gpsimd.tensor_tensor(
                d_bf[t % 2], xr_sb[:, t, :], rl_sb[:, t, :], ALU.subtract
            )
            nc.vector.affine_mul_reduce(
                out=junk_bf[:, 0:F], accum_out=recon_parts[:, t : t + 1],
                in0=d_bf[t % 2], in1=d_bf[t % 2], scale=1.0, bias=0.0,
            )

        # ---- G2 pass: cols [3584:5120] (packed for t>=4); chunks 1-6 ----
        for t in range(RT):
            sim_act(t, au, 2)
            au += 1
            # chunk bursts: k1 at t0-1, k2 at t2-3, k3 at t4-5, k4 at t6-7
            k = 1 + t // 2
            for tt in range(4 * (t % 2), 4 * (t % 2) + 4):
                colsum_mm(k, tt)
            if t % 2 == 1:
                colsum_drain(k)
            # chunk 6 burst at t=4 (tiles 0-3 G2 acts done)
            if t == 4:
                for tt in range(4):
                    colsum_mm(6, tt)
                colsum_drain(6)
            # chunk 5 chase with lag 1: after act(t), cols [3584:4096] of tile t-1
            if t >= 1:
                colsum_mm(5, t - 1)

        # epilogue: chunk 5 tail
        colsum_mm(5, 7)
        colsum_drain(5)

        # final reductions + outputs
        nc.vector.reduce_sum(rowsum_sb, rparts, axis=AX.X)
        nc.vector.reduce_sum(stats4[:, 0:1], recon_parts, axis=AX.X)

        nc.sync.dma_start(rowsum_o, rowsum_sb)
        nc.sync.dma_start(possum_o, possum_sb)
        nc.sync.dma_start(partials_o, stats4)
        for k, (ck, ntiles, part) in enumerate(COL_CHUNKS):
            nc.sync.dma_start(colsum_o[k], colstage[part : part + 1, k, :])


def _build():
    nc = bacc.Bacc("TRN2", target_bir_lowering=False, debug=False, num_devices=NCORES)
    proj = nc.dram_tensor("proj", [PROJ_ROWS, D], bf16, kind="ExternalInput").ap()
    xr = nc.dram_tensor("xr", [RPC, F], bf16, kind="ExternalInput").ap()
    rl = nc.dram_tensor("rl", [RPC, F], bf16, kind="ExternalInput").ap()
    ident = nc.dram_tensor("ident", [128, 128], bf16, kind="ExternalInput").ap()
    mask = nc.dram_tensor("mask", [128, 128], bf16, kind="ExternalInput").ap()
    rowsum_o = nc.dram_tensor("rowsum_o", [128, RT], f32, kind="ExternalOutput").ap()
    possum_o = nc.dram_tensor("possum_o", [128, RT], f32, kind="ExternalOutput").ap()
    colsum_o = nc.dram_tensor("colsum_o", [7, 512], f32, kind="ExternalOutput").ap()
    partials_o = nc.dram_tensor("partials_o", [128, 4], f32, kind="ExternalOutput").ap()

    with tile.TileContext(nc) as tc:
        _kernel_body(tc, proj, xr, rl, ident, mask, rowsum_o, possum_o, colsum_o, partials_o)
    nc.compile()
    return nc


_NC_CACHE = None


def _get_nc():
    global _NC_CACHE
    if _NC_CACHE is None:
        _NC_CACHE = _build()
    return _NC_CACHE


def _run(projections, xrecon, recon_label, trace=False, **spmd_kwargs):
    nc = _get_nc()
    P = np.ascontiguousarray(np.asarray(projections, dtype=np.float32))
    XR = np.asarray(xrecon, dtype=np.float32).astype(BF)
    RL = np.asarray(recon_label, dtype=np.float32).astype(BF)
    ident = np.eye(128, dtype=BF)
    mask = np.kron(np.eye(32, dtype=np.float32), np.ones((4, 4), np.float32)).astype(BF)
    in_maps = []
    for c in range(NCORES):
        ploc = np.roll(P, -c * RPC, axis=0)[:PROJ_ROWS].astype(BF)
        in_maps.append(
            {
                "proj": np.ascontiguousarray(ploc),
                "xr": np.ascontiguousarray(XR[c * RPC : (c + 1) * RPC]),
                "rl": np.ascontiguousarray(RL[c * RPC : (c + 1) * RPC]),
                "ident": ident,
                "mask": mask,
            }
        )
    return run_bass_kernel_spmd(
        nc, in_maps, core_ids=list(range(NCORES)), trace=trace, **spmd_kwargs
    )


def _combine(results):
    rowsum = np.zeros(N, np.float64)
    possum = np.zeros(N, np.float64)
    for c in range(NCORES):
        base = c * RPC
        rowsum[base : base + RPC] += results[c]["rowsum_o"].T.reshape(-1).astype(np.float64)
        possum[base : base + RPC] += results[c]["possum_o"].T.reshape(-1).astype(np.float64)
        cs = results[c]["colsum_o"].astype(np.float64).reshape(7, 512)
        for k, (ck, ntiles, part) in enumerate(COL_CHUNKS):
            gidx = (base + ck + np.arange(512)) % N
            rowsum[gidx] += cs[k]
    recon_ss = 0.0
    A = 0.0
    B = 0.0
    for c in range(NCORES):
        p = results[c]["partials_o"].astype(np.float64)
        recon_ss += p[:, 0].sum()
        A += p[:, 1].sum()
        B += p[:, 2].sum()
    closs = float(np.mean(np.log(rowsum) - np.log(possum)))
    recon_loss = recon_ss / (N * F)
    dist_loss = (4.0 * A - B) / ((N // 4) * 6 * D)
    loss = closs + recon_loss + dist_loss
    return (
        np.float32(loss),
        np.float32(closs),
        np.float32(recon_loss),
        np.float32(dist_loss),
    )


def kernel(projections, xrecon, recon_label):
    br = _run(projections, xrecon, recon_label)
    return _combine(br.results)


# revision 34
# speedup vs baseline: 1.2522x; 1.0483x over previous
"""Trainium2 Bass kernel for nn_JointLoss (recon MSE + SimCLR contrastive + group distance loss).

Strategy (symmetric data-parallel over 8 NeuronCores):
  - exp(sim) dominates: ACT is the only engine with exp, 1 col/cycle @1.2GHz.
    The sim matrix is symmetric: each core evaluates its 1024 rows vs column
    groups c..c+3 (full) plus 3 quarters of the c+4 pair block -- 38,912 cols
    instead of 65,536.  Transposed entries are recovered as COLUMN sums of
    evaluated blocks (ones-vector matmuls on the PE accumulating in PSUM
    bank-7 chains), combined on the host.
  - The host pre-transposes projections into the fp8e4m3 DoubleRow layout
    [64, 2, 5120] (x16 scale), so there are no on-chip transposes and sim
    matmuls run at 0.5 cyc/col.  exp scale absorbs the 1/256.
  - 24 activations (PSUM regions A/B = [0:1792]/[1792:3584] ping-pong) write
    exp to SBUF bf16; G0/G1 activations carry accum_out row-sums (DVE
    reduce-accum ops crash on this HW in bf16); G2 row-sums are DVE reduces.
  - GpSimd does the recon-MSE subtract+square; DVE reduces them; DVE also
    does possum (masked diag sums), distance-loss partials, colsum drains.
  - Host finishes in float64 from tiny outputs.
"""

import os
import sys

if "/opt/trn_rl_repo" not in sys.path:
    sys.path.insert(0, "/opt/trn_rl_repo")

from contextlib import ExitStack

import numpy as np
import ml_dtypes

import concourse.bacc as bacc
import concourse.tile as tile
from concourse import mybir
from concourse.bass_utils import run_bass_kernel_spmd

N = 8192
D = 128
F = 784
NCORES = 8
RPC = N // NCORES   # 1024 rows per core
RT = RPC // 128     # 8 row tiles
PROJ_ROWS = 5 * RPC  # 5120: groups c..c+4
TAU = 0.1
FP8_SCALE = 16.0

f32 = mybir.dt.float32
bf16 = mybir.dt.bfloat16
fp8 = mybir.dt.float8e4
BF = ml_dtypes.bfloat16
F8 = ml_dtypes.float8_e4m3fn

# colsum chunks: (local col start, n tiles summed, bank-7 partition)
COL_CHUNKS = [
    (1024, 8, 0),
    (1536, 8, 32),
    (2048, 8, 64),
    (2560, 8, 0),
    (3072, 8, 32),
    (3584, 8, 64),
    (4608, 4, 32),
]

GB0, GB1, GB2 = 1792, 3584, 5120

# G1-pass per-step colsum mms: chunk0 1/step, chunk1 lag-1, chunk2 lag-2
G1_COLS = {t: [(0, t)] + ([(1, t - 1)] if t >= 1 else []) + ([(2, t - 2)] if t >= 2 else [])
           for t in range(RT)}
# G2-pass actions per step: ("mm", chunk, tile) / ("drain", chunk).  Ordered so
# a bank-7 partition is always drained between its chunks' chains.
G2_ACTS = {
    0: [("drain", 0), ("mm", 3, 0), ("mm", 2, 6)],
    1: [("mm", 3, 1), ("mm", 1, 7), ("drain", 1), ("mm", 4, 0), ("mm", 4, 1)],
    2: [("mm", 3, 2), ("mm", 2, 7), ("drain", 2), ("mm", 4, 2), ("mm", 4, 3), ("mm", 5, 0)],
    3: [("mm", 3, 3), ("mm", 4, 4), ("mm", 4, 5), ("mm", 5, 1)],
    4: [("mm", 3, 4), ("mm", 4, 6), ("mm", 4, 7), ("drain", 4), ("mm", 5, 2)],
    5: [("mm", 3, 5), ("mm", 5, 3)],
    6: [("mm", 3, 6), ("mm", 6, 0), ("mm", 6, 1), ("mm", 5, 4)],
    7: [("mm", 3, 7), ("mm", 6, 2), ("mm", 6, 3), ("mm", 5, 5)],
}


def _bank_splits(a, b):
    cuts = [a]
    nxt = (a // 512 + 1) * 512
    while nxt < b:
        cuts.append(nxt)
        nxt += 512
    cuts.append(b)
    return list(zip(cuts[:-1], cuts[1:]))


def _kernel_body(tc, ptdr, pb16, xr, rl, mask, rowsum_o, possum_o, colsum_o, partials_o):
    nc = tc.nc
    AX = mybir.AxisListType
    ALU = mybir.AluOpType
    EXP = mybir.ActivationFunctionType.Exp
    DR = mybir.MatmulPerfMode.DoubleRow
    with ExitStack() as ctx:
        consts = ctx.enter_context(tc.tile_pool(name="consts", bufs=1))
        big = ctx.enter_context(tc.tile_pool(name="big", bufs=1))
        stats = ctx.enter_context(tc.tile_pool(name="stats", bufs=1))
        psum = ctx.enter_context(tc.tile_pool(name="psum", bufs=1, space="PSUM"))

        pt_dr = big.tile([64, 2, PROJ_ROWS], fp8)
        # column-chunk DMAs so early columns land first
        nc.sync.dma_start(pt_dr[:, :, 0:GB0], ptdr[:, :, 0:GB0])
        nc.sync.dma_start(pt_dr[:, :, GB0:GB1], ptdr[:, :, GB0:GB1])
        nc.sync.dma_start(pt_dr[:, :, GB1:GB2], ptdr[:, :, GB1:GB2])
        pb = consts.tile([128, RPC], bf16)
        nc.sync.dma_start(pb, pb16)
        maskb = consts.tile([128, 128], bf16)
        nc.sync.dma_start(maskb, mask)

        exp_sbuf = big.tile([128, RT, PROJ_ROWS], bf16)
        xr_sb = big.tile([128, RT, F], bf16)
        rl_sb = big.tile([128, RT, F], bf16)
        # halves so MSE can start after the first pair of transfers
        nc.sync.dma_start(
            xr_sb[:, 0:4, :], xr[0:512].rearrange("(t p) j -> p t j", p=128)
        )
        nc.sync.dma_start(
            rl_sb[:, 0:4, :], rl[0:512].rearrange("(t p) j -> p t j", p=128)
        )
        nc.sync.dma_start(
            xr_sb[:, 4:8, :], xr[512:1024].rearrange("(t p) j -> p t j", p=128)
        )
        nc.sync.dma_start(
            rl_sb[:, 4:8, :], rl[512:1024].rearrange("(t p) j -> p t j", p=128)
        )

        ones_bf = consts.tile([128, 1], bf16)
        nc.vector.memset(ones_bf, 1.0)

        rparts = stats.tile([128, RT, 3], f32)
        rowsum_sb = stats.tile([128, RT], f32)
        possum_sb = stats.tile([128, RT], f32)
        recon_parts = stats.tile([128, RT], f32)
        stats4 = stats.tile([128, 4], f32)
        s_groups = stats.tile([128, RPC // 4], f32)
        junk_bf = stats.tile([128, RPC], bf16)
        junkf = stats.tile([128, RPC // 4], f32)
        d_bf0 = stats.tile([128, F], bf16)
        d_bf1 = stats.tile([128, F], bf16)
        d2_bf0 = stats.tile([128, F], bf16)
        d2_bf1 = stats.tile([128, F], bf16)
        d_bf = [d_bf0, d_bf1]
        d2_bf = [d2_bf0, d2_bf1]
        pj = stats.tile([128, 128], bf16)
        colstage = stats.tile([128, 7, 512], f32)

        pacc = psum.tile([128, 4096], f32)
        REG = [0, 1792]

        def colsum_mm(k, t):
            ck, ntiles, part = COL_CHUNKS[k]
            nc.tensor.matmul(
                pacc[part : part + 1, 3584:4096],
                ones_bf[:, 0:1],
                exp_sbuf[:, t, ck : ck + 512],
                start=(t == 0),
                stop=(t == ntiles - 1),
            )

        def colsum_drain(k):
            part = COL_CHUNKS[k][2]
            nc.vector.tensor_copy(
                colstage[part : part + 1, k, :], pacc[part : part + 1, 3584:4096]
            )
            nc.sync.dma_start(colsum_o[k], colstage[part : part + 1, k, :])

        def sim_act(t, au, g):
            reg = REG[au % 2]
            w = pt_dr[:, :, t * 128 : (t + 1) * 128]
            if g == 0:
                cols = [(0, GB0)]
            elif g == 1:
                cols = [(GB0, GB1)]
            elif t < 4:
                cols = [(GB1, GB2)]
            else:
                cols = [(GB1, 4096), (4608, 5120)]
            off = 0
            for c0, c1 in cols:
                for p0, p1 in _bank_splits(reg + off, reg + off + (c1 - c0)):
                    s0 = c0 + (p0 - reg - off)
                    nc.tensor.matmul(
                        pacc[:, p0:p1],
                        w,
                        pt_dr[:, :, s0 : s0 + (p1 - p0)],
                        start=True,
                        stop=True,
                        perf_mode=DR,
                    )
                off += c1 - c0
            if g == 0:
                o0, o1 = 0, GB0
            elif g == 1:
                o0, o1 = GB0, GB1
            elif t < 4:
                o0, o1 = GB1, GB2
            else:
                o0, o1 = GB1, 4608
            if g < 2:
                nc.scalar.activation(
                    exp_sbuf[:, t, o0:o1],
                    pacc[:, reg : reg + (o1 - o0)],
                    EXP,
                    scale=1.0 / (TAU * FP8_SCALE * FP8_SCALE),
                    accum_out=rparts[:, t, g : g + 1],
                )
            else:
                nc.scalar.activation(
                    exp_sbuf[:, t, o0:o1],
                    pacc[:, reg : reg + (o1 - o0)],
                    EXP,
                    scale=1.0 / (TAU * FP8_SCALE * FP8_SCALE),
                )
                nc.vector.reduce_sum(
                    rparts[:, t, 2:3], exp_sbuf[:, t, o0:o1], axis=AX.X
                )

        # distance-loss partials (DVE) from own rows
        nc.vector.reduce_sum(s_groups, pb.rearrange("p (g s) -> p g s", s=4), axis=AX.X)
        nc.vector.affine_mul_reduce(
            out=junk_bf, accum_out=stats4[:, 1:2], in0=pb, in1=pb, scale=1.0, bias=0.0
        )
        nc.vector.tensor_tensor(junkf, s_groups, s_groups, ALU.mult)
        nc.vector.reduce_sum(stats4[:, 2:3], junkf, axis=AX.X)
        nc.vector.memset(stats4[:, 3:4], 0.0)

        au = 0

        # ---- G0 pass: cols [0:1792] ----
        for t in range(RT):
            sim_act(t, au, 0)
            au += 1
            nc.vector.tensor_tensor(
                pj, exp_sbuf[:, t, t * 128 : (t + 1) * 128], maskb, ALU.mult
            )
            nc.vector.reduce_sum(possum_sb[:, t : t + 1], pj, axis=AX.X)

        # ---- G1 pass: cols [1792:3584]; colsum chunks 0-2 chase; MSE ----
        for t in range(RT):
            sim_act(t, au, 1)
            au += 1
            for k, tt in G1_COLS[t]:
                colsum_mm(k, tt)
            # MSE tile t on Pool (sub then square), DVE reduce
            nc.gpsimd.tensor_tensor(
                d_bf[t % 2], xr_sb[:, t, :], rl_sb[:, t, :], ALU.subtract
            )
            nc.gpsimd.tensor_tensor(
                d2_bf[t % 2], d_bf[t % 2], d_bf[t % 2], ALU.mult
            )
            nc.vector.reduce_sum(recon_parts[:, t : t + 1], d2_bf[t % 2], axis=AX.X)

        # ---- G2 pass: cols [3584:5120] (packed for t>=4); chunks finish ----
        for t in range(RT):
            sim_act(t, au, 2)
            au += 1
            for act in G2_ACTS[t]:
                if act[0] == "mm":
                    colsum_mm(act[1], act[2])
                else:
                    colsum_drain(act[1])

        # epilogue
        colsum_mm(5, 6)
        colsum_mm(5, 7)
        colsum_drain(5)
        colsum_drain(3)
        colsum_drain(6)

        nc.vector.reduce_sum(rowsum_sb, rparts, axis=AX.X)
        nc.vector.reduce_sum(stats4[:, 0:1], recon_parts, axis=AX.X)

        nc.sync.dma_start(rowsum_o, rowsum_sb)
        nc.sync.dma_start(possum_o, possum_sb)
        nc.sync.dma_start(partials_o, stats4)


def _build():
    nc = bacc.Bacc("TRN2", target_bir_lowering=False, debug=False, num_devices=NCORES)
    ptdr = nc.dram_tensor("ptdr", [64, 2, PROJ_ROWS], fp8, kind="ExternalInput").ap()
    pb16 = nc.dram_tensor("pb16", [128, RPC], bf16, kind="ExternalInput").ap()
    xr = nc.dram_tensor("xr", [RPC, F], bf16, kind="ExternalInput").ap()
    rl = nc.dram_tensor("rl", [RPC, F], bf16, kind="ExternalInput").ap()
    mask = nc.dram_tensor("mask", [128, 128], bf16, kind="ExternalInput").ap()
    rowsum_o = nc.dram_tensor("rowsum_o", [128, RT], f32, kind="ExternalOutput").ap()
    possum_o = nc.dram_tensor("possum_o", [128, RT], f32, kind="ExternalOutput").ap()
    colsum_o = nc.dram_tensor("colsum_o", [7, 512], f32, kind="ExternalOutput").ap()
    partials_o = nc.dram_tensor("partials_o", [128, 4], f32, kind="ExternalOutput").ap()

    with tile.TileContext(nc) as tc:
        _kernel_body(tc, ptdr, pb16, xr, rl, mask, rowsum_o, possum_o, colsum_o, partials_o)
    nc.compile()
    return nc


_NC_CACHE = None


def _get_nc():
    global _NC_CACHE
    if _NC_CACHE is None:
        _NC_CACHE = _build()
    return _NC_CACHE


def _prep_core(P, XR, RL, c):
    ploc = np.roll(P, -c * RPC, axis=0)[:PROJ_ROWS]  # [5120, 128] f32
    # fp8 DoubleRow layout: ptdr[p, i, j] = 16 * ploc[j, 2p+i]
    pt = (ploc.T * FP8_SCALE).astype(F8)  # [128, 5120]
    ptdr = np.ascontiguousarray(pt.reshape(64, 2, PROJ_ROWS))
    pb16 = np.ascontiguousarray(ploc[:RPC].T.astype(BF))  # [128, 1024]
    return ptdr, pb16


def _run(projections, xrecon, recon_label, trace=False, **spmd_kwargs):
    nc = _get_nc()
    P = np.ascontiguousarray(np.asarray(projections, dtype=np.float32))
    XR = np.asarray(xrecon, dtype=np.float32).astype(BF)
    RL = np.asarray(recon_label, dtype=np.float32).astype(BF)
    mask = np.kron(np.eye(32, dtype=np.float32), np.ones((4, 4), np.float32)).astype(BF)
    in_maps = []
    for c in range(NCORES):
        ptdr, pb16 = _prep_core(P, XR, RL, c)
        in_maps.append(
            {
                "ptdr": ptdr,
                "pb16": pb16,
                "xr": np.ascontiguousarray(XR[c * RPC : (c + 1) * RPC]),
                "rl": np.ascontiguousarray(RL[c * RPC : (c + 1) * RPC]),
                "mask": mask,
            }
        )
    return run_bass_kernel_spmd(
        nc, in_maps, core_ids=list(range(NCORES)), trace=trace, **spmd_kwargs
    )


def _combine(results):
    rowsum = np.zeros(N, np.float64)
    possum = np.zeros(N, np.float64)
    for c in range(NCORES):
        base = c * RPC
        rowsum[base : base + RPC] += results[c]["rowsum_o"].T.reshape(-1).astype(np.float64)
        possum[base : base + RPC] += results[c]["possum_o"].T.reshape(-1).astype(np.float64)
        cs = results[c]["colsum_o"].astype(np.float64).reshape(7, 512)
        for k, (ck, ntiles, part) in enumerate(COL_CHUNKS):
            gidx = (base + ck + np.arange(512)) % N
            rowsum[gidx] += cs[k]
    recon_ss = 0.0
    A = 0.0
    B = 0.0
    for c in range(NCORES):
        p = results[c]["partials_o"].astype(np.float64)
        recon_ss += p[:, 0].sum()
        A += p[:, 1].sum()
        B += p[:, 2].sum()
    closs = float(np.mean(np.log(rowsum) - np.log(possum)))
    recon_loss = recon_ss / (N * F)
    dist_loss = (4.0 * A - B) / ((N // 4) * 6 * D)
    loss = closs + recon_loss + dist_loss
    return (
        np.float32(loss),
        np.float32(closs),
        np.float32(recon_loss),
        np.float32(dist_loss),
    )


def kernel(projections, xrecon, recon_label):
    br = _run(projections, xrecon, recon_label)
    return _combine(br.results)
